# revision 1
# baseline (speedup 1.0000x reference)
import os
import sys
import numpy as np
from contextlib import ExitStack

for _p in ("/opt/trn_rl_repo", "/root/.axon_site/_ro/trn_rl_repo"):
    if os.path.isdir(_p) and _p not in sys.path:
        sys.path.append(_p)

D = 256
H = 4
DH = 64
N_SRC = 100000
N_DST = 50000
N_EDGES = 300000
NDEV = 8
DST_PER_DEV = N_DST // NDEV  # 6250
NBLK = (DST_PER_DEV + 127) // 128  # 49
DST_PAD = NBLK * 128  # 6272

LAST_EXEC_NS = None


def _prep_host(h_src, h_dst, src_idx, dst_idx, Wq, bq, Wk, bk, Wv, bv):
    order = np.argsort(dst_idx, kind="stable")
    sdst = dst_idx[order]
    bounds = np.searchsorted(sdst, np.arange(0, N_DST + 1, DST_PER_DEV))

    per_dev = []
    C = 1
    for d in range(NDEV):
        lo, hi = int(bounds[d]), int(bounds[d + 1])
        local = (sdst[lo:hi] - d * DST_PER_DEV).astype(np.int64)
        blk = local // 128
        cnt = np.bincount(blk, minlength=NBLK)
        if cnt.max() > 0:
            C = max(C, int(np.ceil(cnt.max() / 128.0)))
        per_dev.append((lo, hi, local, blk, cnt))

    WKV = np.ascontiguousarray(
        np.concatenate([Wk.T, Wv.T], axis=1).astype(np.float32).reshape(2, 128, 512))
    WQ = np.ascontiguousarray(Wq.T.astype(np.float32).reshape(2, 128, 256))
    BKV = np.concatenate([bk, bv]).astype(np.float32).reshape(1, 512)
    BQ = bq.astype(np.float32).reshape(1, 256)
    has_bias = bool(np.any(BKV) or np.any(BQ))

    nchunks = NBLK * C
    E_pad = nchunks * 128
    in_maps = []
    for d in range(NDEV):
        lo, hi, local, blk, cnt = per_dev[d]
        starts = np.concatenate([[0], np.cumsum(cnt)[:-1]])
        pos = np.arange(hi - lo) - starts[blk]
        slot = blk * (C * 128) + pos

        eids = order[lo:hi]
        Xf = np.zeros((E_pad, D), np.float32)
        Xf[slot] = h_src[src_idx[eids]]
        dloc = np.full(E_pad, 128, np.int64)
        dloc[slot] = local % 128
        A2f = np.zeros((E_pad, 129), np.float32)
        A2f[np.arange(E_pad), dloc] = 1.0
        A2 = np.ascontiguousarray(A2f[:, :128].reshape(nchunks, 128, 128))
        A1 = np.ascontiguousarray(A2.transpose(0, 2, 1))
        X = np.ascontiguousarray(Xf.reshape(nchunks, 128, D).transpose(0, 2, 1))

        hd = np.zeros((DST_PAD, D), np.float32)
        hd[:DST_PER_DEV] = h_dst[d * DST_PER_DEV:(d + 1) * DST_PER_DEV]
        HD = np.ascontiguousarray(hd.reshape(NBLK, 128, 2, 128).transpose(0, 2, 3, 1))

        in_maps.append({"X": X, "A1": A1, "A2": A2, "HD": HD,
                        "WKV": WKV, "WQ": WQ, "BKV": BKV, "BQ": BQ})
    return in_maps, C, has_bias


def _build(C, has_bias):
    from concourse import bacc, bass, mybir, tile

    F32 = mybir.dt.float32
    nchunks = NBLK * C
    nc = bacc.Bacc(trn_type="TRN2")
    X_d = nc.dram_tensor("X", [nchunks, D, 128], F32, kind="ExternalInput")
    A1_d = nc.dram_tensor("A1", [nchunks, 128, 128], F32, kind="ExternalInput")
    A2_d = nc.dram_tensor("A2", [nchunks, 128, 128], F32, kind="ExternalInput")
    HD_d = nc.dram_tensor("HD", [NBLK, 2, 128, 128], F32, kind="ExternalInput")
    WKV_d = nc.dram_tensor("WKV", [2, 128, 512], F32, kind="ExternalInput")
    WQ_d = nc.dram_tensor("WQ", [2, 128, 256], F32, kind="ExternalInput")
    BKV_d = nc.dram_tensor("BKV", [1, 512], F32, kind="ExternalInput")
    BQ_d = nc.dram_tensor("BQ", [1, 256], F32, kind="ExternalInput")
    out_d = nc.dram_tensor("out", [NBLK, 128, 256], F32, kind="ExternalOutput")

    Copy = mybir.ActivationFunctionType.Copy
    Exp = mybir.ActivationFunctionType.Exp
    mult = mybir.AluOpType.mult
    addop = mybir.AluOpType.add
    maxop = mybir.AluOpType.max

    with ExitStack() as ctx:
        tc = ctx.enter_context(tile.TileContext(nc))
        cpool = ctx.enter_context(tc.tile_pool(name="const", bufs=1))
        bpool = ctx.enter_context(tc.tile_pool(name="blk", bufs=2))
        kpool = ctx.enter_context(tc.tile_pool(name="chunk", bufs=3))
        qpp = ctx.enter_context(tc.tile_pool(name="qps", bufs=1, space="PSUM"))
        upp = ctx.enter_context(tc.tile_pool(name="ups", bufs=2, space="PSUM"))
        kpp = ctx.enter_context(tc.tile_pool(name="kvp", bufs=2, space="PSUM"))
        gpp = ctx.enter_context(tc.tile_pool(name="qgp", bufs=2, space="PSUM"))

        wkv_sb = cpool.tile([128, 2, 512], F32)
        nc.sync.dma_start(out=wkv_sb, in_=WKV_d.rearrange("s p e -> p s e"))
        wq_sb = cpool.tile([128, 2, 256], F32)
        nc.sync.dma_start(out=wq_sb, in_=WQ_d.rearrange("s p e -> p s e"))
        if has_bias:
            ones_sb = cpool.tile([1, 128], F32)
            nc.vector.memset(ones_sb, 1.0)
            bkv_sb = cpool.tile([1, 512], F32)
            nc.sync.dma_start(out=bkv_sb, in_=BKV_d)
            bq_sb = cpool.tile([1, 256], F32)
            nc.sync.dma_start(out=bq_sb, in_=BQ_d)

        for b in range(NBLK):
            hd_sb = bpool.tile([128, 2, 128], F32)
            nc.sync.dma_start(out=hd_sb, in_=HD_d[b].rearrange("s c d -> c s d"))
            xblk = bpool.tile([128, C, 2, 128], F32)
            nc.sync.dma_start(
                out=xblk,
                in_=X_d[b * C:(b + 1) * C].rearrange("c (s p) e -> p c s e", s=2))
            a1 = bpool.tile([128, C, 128], F32)
            nc.sync.dma_start(out=a1, in_=A1_d[b * C:(b + 1) * C].rearrange("c p e -> p c e"))
            a2 = bpool.tile([128, C, 128], F32)
            nc.sync.dma_start(out=a2, in_=A2_d[b * C:(b + 1) * C].rearrange("c p e -> p c e"))

            qps = qpp.tile([128, 256], F32)
            nc.tensor.matmul(qps, hd_sb[:, 0, :], wq_sb[:, 0, :],
                             start=True, stop=False)
            nc.tensor.matmul(qps, hd_sb[:, 1, :], wq_sb[:, 1, :],
                             start=False, stop=not has_bias)
            if has_bias:
                nc.tensor.matmul(qps, ones_sb, bq_sb, start=False, stop=True)
            q_sb = bpool.tile([128, 256], F32)
            nc.scalar.activation(q_sb, qps, Copy)

            ups = upp.tile([128, 260], F32)
            for c in range(C):
                kv = kpp.tile([128, 512], F32)
                nc.tensor.matmul(kv, xblk[:, c, 0, :], wkv_sb[:, 0, :],
                                 start=True, stop=False)
                nc.tensor.matmul(kv, xblk[:, c, 1, :], wkv_sb[:, 1, :],
                                 start=False, stop=not has_bias)
                if has_bias:
                    nc.tensor.matmul(kv, ones_sb, bkv_sb, start=False, stop=True)
                qg = gpp.tile([128, 256], F32)
                nc.tensor.matmul(qg, a1[:, c, :], q_sb, start=True, stop=True)
                qg_sb = kpool.tile([128, 256], F32)
                nc.scalar.activation(qg_sb, qg, Copy)
                prod = kpool.tile([128, 256], F32)
                nc.vector.tensor_tensor(prod, kv[:, 0:256], qg_sb, mult)
                sc = kpool.tile([128, 4], F32)
                nc.vector.tensor_reduce(sc, prod.rearrange("p (h d) -> p h d", h=4),
                                        mybir.AxisListType.X, addop)
                es = kpool.tile([128, 4], F32)
                nc.scalar.activation(es, sc, Exp, scale=0.125)
                pcat = kpool.tile([128, 260], F32)
                nc.vector.tensor_scalar(pcat[:, 256:260], es, 0.0, None, addop)
                for h in range(H):
                    nc.vector.tensor_scalar(
                        pcat[:, h * 64:(h + 1) * 64],
                        kv[:, 256 + h * 64:256 + (h + 1) * 64],
                        es[:, h:h + 1], None, mult)
                nc.tensor.matmul(ups, a2[:, c, :], pcat,
                                 start=(c == 0), stop=(c == C - 1))

            s_sb = bpool.tile([128, 4], F32)
            nc.vector.tensor_scalar(s_sb, ups[:, 256:260], 1e-30, None, maxop)
            r_sb = bpool.tile([128, 4], F32)
            nc.vector.reciprocal(r_sb, s_sb)
            o_sb = bpool.tile([128, 256], F32)
            for h in range(H):
                nc.vector.tensor_scalar(o_sb[:, h * 64:(h + 1) * 64],
                                        ups[:, h * 64:(h + 1) * 64],
                                        r_sb[:, h:h + 1], None, mult)
            nc.sync.dma_start(out=out_d[b], in_=o_sb)
    return nc


def _emulate(in_maps, C):
    outs = []
    for m in in_maps:
        X, A1, A2, HD = m["X"], m["A1"], m["A2"], m["HD"]
        WKV, WQ, BKV, BQ = m["WKV"], m["WQ"], m["BKV"], m["BQ"]
        out = np.zeros((NBLK, 128, 256), np.float32)
        for b in range(NBLK):
            hd = HD[b]
            Q = hd[0].T @ WQ[0] + hd[1].T @ WQ[1] + BQ
            U = np.zeros((128, 260), np.float32)
            for c in range(C):
                i = b * C + c
                x = X[i]
                kv = x[:128].T @ WKV[0] + x[128:].T @ WKV[1] + BKV
                qg = A1[i].T @ Q
                sc = (kv[:, :256] * qg).reshape(128, 4, 64).sum(-1)
                p = np.exp(sc * 0.125).astype(np.float32)
                pv = (kv[:, 256:].reshape(128, 4, 64) * p[:, :, None]).reshape(128, 256)
                U += A2[i].T @ np.concatenate([pv, p], axis=1)
            r = 1.0 / np.maximum(U[:, 256:260], 1e-30)
            out[b] = (U[:, :256].reshape(128, 4, 64) * r[:, :, None]).reshape(128, 256)
        outs.append({"out": out})
    return outs


def kernel(**inputs):
    global LAST_EXEC_NS
    h_src = np.asarray(inputs["h_src"], np.float32)
    h_dst = np.asarray(inputs["h_dst"], np.float32)
    src_idx = np.asarray(inputs["src_idx"]).astype(np.int64)
    dst_idx = np.asarray(inputs["dst_idx"]).astype(np.int64)
    Wq = np.asarray(inputs["Wq"], np.float32)
    bq = np.asarray(inputs["bq"], np.float32)
    Wk = np.asarray(inputs["Wk"], np.float32)
    bk = np.asarray(inputs["bk"], np.float32)
    Wv = np.asarray(inputs["Wv"], np.float32)
    bv = np.asarray(inputs["bv"], np.float32)

    in_maps, C, has_bias = _prep_host(h_src, h_dst, src_idx, dst_idx,
                                      Wq, bq, Wk, bk, Wv, bv)

    if os.environ.get("KERNEL_EMULATE"):
        results = _emulate(in_maps, C)
    else:
        from concourse.bass_utils import run_bass_kernel_spmd
        nc = _build(C, has_bias)
        nc.finalize()
        res = run_bass_kernel_spmd(
            nc, in_maps, core_ids=list(range(NDEV)),
            trace=bool(os.environ.get("KERNEL_TRACE")))
        results = res.results
        LAST_EXEC_NS = res.exec_time_ns

    parts = [np.asarray(r["out"]).reshape(DST_PAD, 256)[:DST_PER_DEV]
             for r in results]
    return np.ascontiguousarray(np.concatenate(parts, axis=0).astype(np.float32))



# revision 2
# speedup vs baseline: 7.1865x; 7.1865x over previous
import os
import sys
import numpy as np
from contextlib import ExitStack

for _p in ("/opt/trn_rl_repo", "/root/.axon_site/_ro/trn_rl_repo"):
    if os.path.isdir(_p) and _p not in sys.path:
        sys.path.append(_p)

import ml_dtypes

BF16 = ml_dtypes.bfloat16

D = 256
H = 4
DH = 64
N_SRC = 100000
N_DST = 50000
N_EDGES = 300000
NDEV = 8
DST_PER_DEV = N_DST // NDEV  # 6250
NBLK = (DST_PER_DEV + 127) // 128  # 49
DST_PAD = NBLK * 128  # 6272

LAST_EXEC_NS = None


def _wrap16(vals):
    """[G, NI] index values -> [128, G*NI//16] int16 in dma_gather layout:
    gather i lives at partition i%16, column i//16, replicated to 128 rows."""
    G, NI = vals.shape
    w = vals.reshape(G, NI // 16, 16).transpose(2, 0, 1).reshape(16, G * (NI // 16))
    return np.ascontiguousarray(np.tile(w, (8, 1)).astype(np.int16))


def _prep_host(h_src, h_dst, src_idx, dst_idx, Wq, bq, Wk, bk, Wv, bv):
    hsb = h_src.astype(BF16)
    hdb = h_dst.astype(BF16)

    order = np.argsort(dst_idx, kind="stable")
    sdst = dst_idx[order]
    ssrc = src_idx[order]
    bounds = np.searchsorted(sdst, np.arange(0, N_DST + 1, DST_PER_DEV))

    infos = []
    C = 1
    UMAX = 1
    for d in range(NDEV):
        lo, hi = int(bounds[d]), int(bounds[d + 1])
        local = (sdst[lo:hi] - d * DST_PER_DEV).astype(np.int64)
        blk = local >> 7
        cnt = np.bincount(blk, minlength=NBLK)
        if cnt.max() > 0:
            C = max(C, int(np.ceil(cnt.max() / 128.0)))
        uniq, inv = np.unique(ssrc[lo:hi], return_inverse=True)
        UMAX = max(UMAX, len(uniq))
        starts = np.concatenate([[0], np.cumsum(cnt)[:-1]])
        pos = np.arange(hi - lo) - starts[blk]
        infos.append((local, blk, pos, uniq, inv))

    UPAD = UMAX
    assert UPAD <= 32000, UPAD  # dma_gather indices are int16
    NI = C * 128
    NCH = NBLK * C

    XU = np.zeros((NDEV * UPAD, D), BF16)
    HDR = np.zeros((NDEV * DST_PAD, D), BF16)
    XIDX_v = np.zeros((NDEV, NBLK, NI), np.int64)
    QIDX_v = np.zeros((NDEV, NBLK, NI), np.int64)
    DLOC = np.full((NDEV * 128, NCH), 128.0, np.float32)

    for d in range(NDEV):
        local, blk, pos, uniq, inv = infos[d]
        XU[d * UPAD:d * UPAD + len(uniq)] = hsb[uniq]
        HDR[d * DST_PAD:d * DST_PAD + DST_PER_DEV] = \
            hdb[d * DST_PER_DEV:(d + 1) * DST_PER_DEV]
        XIDX_v[d, blk, pos] = inv
        QIDX_v[d, blk, pos] = local
        DLOC[d * 128 + pos % 128, blk * C + pos // 128] = local % 128

    XIDX = np.concatenate([_wrap16(XIDX_v[d]) for d in range(NDEV)], axis=0)
    QIDX = np.concatenate([_wrap16(QIDX_v[d]) for d in range(NDEV)], axis=0)
    hv = (np.arange(NBLK * 128, dtype=np.int64).reshape(NBLK, 128))
    HIDX = np.concatenate([_wrap16(hv)] * NDEV, axis=0)

    wkv = np.concatenate([Wk.T, Wv.T], axis=1).astype(BF16).reshape(2, 128, 2 * D)
    wq = np.ascontiguousarray(Wq.T.astype(BF16).reshape(2, 128, D))
    WKV = np.ascontiguousarray(np.broadcast_to(wkv, (NDEV,) + wkv.shape)
                               .reshape(NDEV * 2, 128, 2 * D))
    WQ = np.ascontiguousarray(np.broadcast_to(wq, (NDEV,) + wq.shape)
                              .reshape(NDEV * 2, 128, D))
    bkv = np.concatenate([bk, bv]).astype(BF16).reshape(1, 2 * D)
    bqr = bq.astype(BF16).reshape(1, D)
    has_bias = bool(np.any(bk) or np.any(bv) or np.any(bq))
    BKV = np.ascontiguousarray(np.broadcast_to(bkv, (NDEV, 1, 2 * D))
                               .reshape(NDEV * 1, 2 * D))
    BQ = np.ascontiguousarray(np.broadcast_to(bqr, (NDEV, 1, D))
                              .reshape(NDEV * 1, D))

    arrays = {"XU": XU, "HDR": HDR, "XIDX": XIDX, "QIDX": QIDX, "HIDX": HIDX,
              "DLOC": DLOC, "WKV": WKV, "WQ": WQ}
    if has_bias:
        arrays["BKV"] = BKV
        arrays["BQ"] = BQ
    return arrays, C, UPAD, has_bias


def _build(C, UPAD, has_bias):
    from concourse import bacc, mybir, tile

    F32 = mybir.dt.float32
    BF = mybir.dt.bfloat16
    I16 = mybir.dt.int16
    I32 = mybir.dt.int32
    Copy = mybir.ActivationFunctionType.Copy
    Exp = mybir.ActivationFunctionType.Exp
    mult = mybir.AluOpType.mult
    addop = mybir.AluOpType.add
    maxop = mybir.AluOpType.max
    iseq = mybir.AluOpType.is_equal

    NI = C * 128
    NCH = NBLK * C

    nc = bacc.Bacc(trn_type="TRN2")
    XU_d = nc.dram_tensor("XU", [UPAD, D], BF, kind="ExternalInput")
    HDR_d = nc.dram_tensor("HDR", [DST_PAD, D], BF, kind="ExternalInput")
    XIDX_d = nc.dram_tensor("XIDX", [128, NBLK * (NI // 16)], I16,
                            kind="ExternalInput")
    QIDX_d = nc.dram_tensor("QIDX", [128, NBLK * (NI // 16)], I16,
                            kind="ExternalInput")
    HIDX_d = nc.dram_tensor("HIDX", [128, NBLK * 8], I16, kind="ExternalInput")
    DLOC_d = nc.dram_tensor("DLOC", [128, NCH], F32, kind="ExternalInput")
    WKV_d = nc.dram_tensor("WKV", [2, 128, 2 * D], BF, kind="ExternalInput")
    WQ_d = nc.dram_tensor("WQ", [2, 128, D], BF, kind="ExternalInput")
    if has_bias:
        BKV_d = nc.dram_tensor("BKV", [1, 2 * D], BF, kind="ExternalInput")
        BQ_d = nc.dram_tensor("BQ", [1, D], BF, kind="ExternalInput")
    out_d = nc.dram_tensor("out", [NBLK, 128, D], BF, kind="ExternalOutput")

    with ExitStack() as ctx:
        tc = ctx.enter_context(tile.TileContext(nc))
        cpool = ctx.enter_context(tc.tile_pool(name="const", bufs=1))
        bpool = ctx.enter_context(tc.tile_pool(name="blk", bufs=2))
        kpool = ctx.enter_context(tc.tile_pool(name="chunk", bufs=3))
        qpp = ctx.enter_context(tc.tile_pool(name="qps", bufs=1, space="PSUM"))
        upp = ctx.enter_context(tc.tile_pool(name="ups", bufs=2, space="PSUM"))
        kpp = ctx.enter_context(tc.tile_pool(name="kvp", bufs=2, space="PSUM"))
        drp = ctx.enter_context(tc.tile_pool(name="qdr", bufs=1, space="DRAM"))

        wkv_sb = cpool.tile([128, 2, 2 * D], BF)
        nc.sync.dma_start(out=wkv_sb, in_=WKV_d.rearrange("s p e -> p s e"))
        wq_sb = cpool.tile([128, 2, D], BF)
        nc.sync.dma_start(out=wq_sb, in_=WQ_d.rearrange("s p e -> p s e"))
        xidx_sb = cpool.tile([128, NBLK * (NI // 16)], I16)
        nc.sync.dma_start(out=xidx_sb, in_=XIDX_d[:])
        qidx_sb = cpool.tile([128, NBLK * (NI // 16)], I16)
        nc.sync.dma_start(out=qidx_sb, in_=QIDX_d[:])
        hidx_sb = cpool.tile([128, NBLK * 8], I16)
        nc.sync.dma_start(out=hidx_sb, in_=HIDX_d[:])
        dloc_sb = cpool.tile([128, NCH], F32)
        nc.sync.dma_start(out=dloc_sb, in_=DLOC_d[:])
        iota_i = cpool.tile([128, 128], I32)
        nc.gpsimd.iota(iota_i, pattern=[[1, 128]], base=0, channel_multiplier=0)
        iota_f = cpool.tile([128, 128], F32)
        nc.vector.tensor_copy(iota_f, iota_i)
        if has_bias:
            ones_sb = cpool.tile([1, 2 * D], BF)
            nc.vector.memset(ones_sb, 1.0)
            bkv_sb = cpool.tile([1, 2 * D], BF)
            nc.sync.dma_start(out=bkv_sb, in_=BKV_d[:])
            bq_sb = cpool.tile([1, D], BF)
            nc.sync.dma_start(out=bq_sb, in_=BQ_d[:])

        q_dr = drp.tile([DST_PAD, D], F32)
        nq16 = NI // 16

        for b in range(NBLK):
            hd_t = bpool.tile([128, 2, 128], BF)
            nc.gpsimd.dma_gather(
                out_ap=hd_t[:], in_ap=HDR_d[:],
                idxs_ap=hidx_sb[:, b * 8:(b + 1) * 8],
                num_idxs=128, num_idxs_reg=128, elem_size=D, transpose=True)
            qps = qpp.tile([128, D], F32)
            nc.tensor.matmul(qps, hd_t[:, 0, :], wq_sb[:, 0, :],
                             start=True, stop=False)
            nc.tensor.matmul(qps, hd_t[:, 1, :], wq_sb[:, 1, :],
                             start=False, stop=not has_bias)
            if has_bias:
                nc.tensor.matmul(qps, ones_sb[:, :D], bq_sb, start=False,
                                 stop=True)
            q_sb = bpool.tile([128, D], F32)
            nc.scalar.activation(q_sb, qps, Copy)
            nc.sync.dma_start(out=q_dr[b * 128:(b + 1) * 128], in_=q_sb)

            xt = bpool.tile([128, 2, NI], BF)
            nc.gpsimd.dma_gather(
                out_ap=xt[:], in_ap=XU_d[:],
                idxs_ap=xidx_sb[:, b * nq16:(b + 1) * nq16],
                num_idxs=NI, num_idxs_reg=NI, elem_size=D, transpose=True)
            qg = bpool.tile([128, C, D], F32)
            nc.gpsimd.dma_gather(
                out_ap=qg[:], in_ap=q_dr[:],
                idxs_ap=qidx_sb[:, b * nq16:(b + 1) * nq16],
                num_idxs=NI, num_idxs_reg=NI, elem_size=D, transpose=False)

            ups = upp.tile([128, D + 4], F32)
            for c in range(C):
                g = b * C + c
                kv = kpp.tile([128, 2 * D], F32)
                nc.tensor.matmul(kv, xt[:, 0, c * 128:(c + 1) * 128],
                                 wkv_sb[:, 0, :], start=True, stop=False)
                nc.tensor.matmul(kv, xt[:, 1, c * 128:(c + 1) * 128],
                                 wkv_sb[:, 1, :], start=False,
                                 stop=not has_bias)
                if has_bias:
                    nc.tensor.matmul(kv, ones_sb, bkv_sb, start=False,
                                     stop=True)
                a2 = kpool.tile([128, 128], F32)
                nc.vector.tensor_scalar(a2, iota_f, dloc_sb[:, g:g + 1], None,
                                        iseq)
                prod = kpool.tile([128, D], F32)
                nc.vector.tensor_tensor(prod, kv[:, 0:D], qg[:, c, :], mult)
                sc = kpool.tile([128, H], F32)
                nc.vector.tensor_reduce(sc,
                                        prod.rearrange("p (h d) -> p h d", h=H),
                                        mybir.AxisListType.X, addop)
                es = kpool.tile([128, H], F32)
                nc.scalar.activation(es, sc, Exp, scale=1.0 / np.sqrt(DH))
                pcat = kpool.tile([128, D + 4], F32)
                nc.vector.tensor_scalar(pcat[:, D:D + 4], es, 0.0, None, addop)
                for h in range(H):
                    nc.vector.tensor_scalar(
                        pcat[:, h * DH:(h + 1) * DH],
                        kv[:, D + h * DH:D + (h + 1) * DH],
                        es[:, h:h + 1], None, mult)
                nc.tensor.matmul(ups, a2, pcat, start=(c == 0),
                                 stop=(c == C - 1))

            s_sb = bpool.tile([128, H], F32)
            nc.vector.tensor_scalar(s_sb, ups[:, D:D + 4], 1e-30, None, maxop)
            r_sb = bpool.tile([128, H], F32)
            nc.vector.reciprocal(r_sb, s_sb)
            o_sb = bpool.tile([128, D], BF)
            for h in range(H):
                nc.vector.tensor_scalar(o_sb[:, h * DH:(h + 1) * DH],
                                        ups[:, h * DH:(h + 1) * DH],
                                        r_sb[:, h:h + 1], None, mult)
            nc.sync.dma_start(out=out_d[b], in_=o_sb)
    return nc


def _run_pjrt(nc, arrays):
    """Mirror of bass2jax.run_bass_via_pjrt's multi-core path, but fed
    pre-concatenated global arrays (axis 0 = core) to avoid extra copies."""
    import jax
    import numpy as _np
    from jax.sharding import Mesh, PartitionSpec
    from jax.experimental.shard_map import shard_map
    from concourse import mybir
    from concourse.bass2jax import (_bass_exec_p, install_neuronx_cc_hook,
                                    partition_id_tensor)

    install_neuronx_cc_hook()

    partition_name = (nc.partition_id_tensor.name
                      if nc.partition_id_tensor else None)
    in_names = []
    out_names = []
    out_avals = []
    zero_outs = []
    for alloc in nc.m.functions[0].allocations:
        if not isinstance(alloc, mybir.MemoryLocationSet):
            continue
        name = alloc.memorylocations[0].name
        if alloc.kind == "ExternalInput":
            if name != partition_name:
                in_names.append(name)
        elif alloc.kind == "ExternalOutput":
            out_names.append(name)
            shape = tuple(alloc.tensor_shape)
            dtype = mybir.dt.np(alloc.dtype)
            out_avals.append(jax.core.ShapedArray(shape, dtype))
            zero_outs.append(_np.zeros((NDEV * shape[0],) + shape[1:], dtype))
    n_params = len(in_names)
    n_outs = len(out_avals)
    all_names = list(in_names) + list(out_names)
    if partition_name is not None:
        all_names.append(partition_name)
    donate = tuple(range(n_params, n_params + n_outs))

    def _body(*args):
        operands = list(args)
        if partition_name is not None:
            operands.append(partition_id_tensor())
        outs = _bass_exec_p.bind(
            *operands,
            out_avals=tuple(out_avals),
            in_names=tuple(all_names),
            out_names=tuple(out_names),
            lowering_input_output_aliases=(),
            sim_require_finite=True,
            sim_require_nnan=True,
            nc=nc,
        )
        return tuple(outs)

    devices = jax.devices()[:NDEV]
    mesh = Mesh(_np.asarray(devices), ("core",))
    in_specs = (PartitionSpec("core"),) * (n_params + n_outs)
    out_specs = (PartitionSpec("core"),) * n_outs
    sharded = jax.jit(
        shard_map(_body, mesh=mesh, in_specs=in_specs, out_specs=out_specs,
                  check_rep=False),
        donate_argnums=donate, keep_unused=True)
    concat_in = [arrays[name] for name in in_names]
    outs = sharded(*concat_in, *zero_outs)
    return [_np.asarray(o) for o in outs], out_names


def _emulate(arrays, C, UPAD, has_bias):
    NI = C * 128
    out_all = np.zeros((NDEV, NBLK, 128, D), np.float32)
    for d in range(NDEV):
        XU = arrays["XU"][d * UPAD:(d + 1) * UPAD].astype(np.float32)
        HDR = arrays["HDR"][d * DST_PAD:(d + 1) * DST_PAD].astype(np.float32)
        wkv = arrays["WKV"][d * 2:(d + 1) * 2].astype(np.float32)
        wq = arrays["WQ"][d * 2:(d + 1) * 2].astype(np.float32)
        dloc = arrays["DLOC"][d * 128:(d + 1) * 128]
        xidxw = arrays["XIDX"][d * 128:(d + 1) * 128]
        qidxw = arrays["QIDX"][d * 128:(d + 1) * 128]
        bkv = (arrays["BKV"][d].astype(np.float32)
               if has_bias else np.zeros(2 * D, np.float32))
        bq = (arrays["BQ"][d].astype(np.float32)
              if has_bias else np.zeros(D, np.float32))

        def unwrap(w, b):
            cols = w[:16, b * (NI // 16):(b + 1) * (NI // 16)]
            return cols.T.reshape(-1)[:NI]

        Q = HDR @ (wq.reshape(D, D)) + bq
        for b in range(NBLK):
            xi = unwrap(xidxw, b)
            qi = unwrap(qidxw, b)
            x = XU[xi]                                   # [NI, 256]
            kvp = (x @ wkv.reshape(D, 2 * D) + bkv)      # [NI, 512]
            qgf = Q[qi]                                  # [NI, 256]
            sc = (kvp[:, :D] * qgf).reshape(NI, H, DH).sum(-1)
            p = np.exp(sc / np.sqrt(DH)).astype(np.float32)
            dl = dloc[:, b * C:(b + 1) * C].T.reshape(NI)  # slot (c,e)->c*128+e
            a2 = (np.arange(128)[None, :] == dl[:, None]).astype(np.float32)
            pv = (kvp[:, D:].reshape(NI, H, DH) * p[:, :, None]).reshape(NI, D)
            ups = a2.T @ np.concatenate([pv, p], axis=1)
            r = 1.0 / np.maximum(ups[:, D:D + 4], 1e-30)
            out_all[d, b] = (ups[:, :D].reshape(128, H, DH)
                             * r[:, :, None]).reshape(128, D)
    return out_all.astype(BF16)


def kernel(**inputs):
    global LAST_EXEC_NS
    LAST_EXEC_NS = None
    h_src = np.asarray(inputs["h_src"], np.float32)
    h_dst = np.asarray(inputs["h_dst"], np.float32)
    src_idx = np.asarray(inputs["src_idx"]).astype(np.int64)
    dst_idx = np.asarray(inputs["dst_idx"]).astype(np.int64)
    Wq = np.asarray(inputs["Wq"], np.float32)
    bq = np.asarray(inputs["bq"], np.float32)
    Wk = np.asarray(inputs["Wk"], np.float32)
    bk = np.asarray(inputs["bk"], np.float32)
    Wv = np.asarray(inputs["Wv"], np.float32)
    bv = np.asarray(inputs["bv"], np.float32)

    arrays, C, UPAD, has_bias = _prep_host(h_src, h_dst, src_idx, dst_idx,
                                           Wq, bq, Wk, bk, Wv, bv)

    if os.environ.get("KERNEL_EMULATE"):
        out = _emulate(arrays, C, UPAD, has_bias)
        out = out.reshape(NDEV * NBLK, 128, D)
    else:
        nc = _build(C, UPAD, has_bias)
        nc.finalize()
        outs, out_names = _run_pjrt(nc, arrays)
        out = outs[out_names.index("out")]

    full = (np.asarray(out).reshape(NDEV, DST_PAD, D)[:, :DST_PER_DEV]
            .reshape(N_DST, D))
    return np.ascontiguousarray(full.astype(np.float32))
